# revision 35
# baseline (speedup 1.0000x reference)
"""Trainium2 Bass kernel for the char-LSTM word-similarity CNN scorer.

Problem: B=8192 examples x NW=4 words x L=16 chars. Per word: char
embeddings -> masked LSTMCell over <=16 steps -> cell state c [128].
Per example: 4x4 cosine matrix of the word reps -> 2-layer 2x2-valid
CNN -> linear scorer -> sigmoid.

Strategy (pure data parallel, 1024 examples / 4096 words per core):
 - Host folds emb @ W_ih.T + (b_ih + b_hh) into a [66, 512] table G65
   (gate order i,f,o,g); per-step char inputs become a K=66 one-hot
   matmul (row 64 = "freeze" flag that drives f->1, i->0 for words past
   their length, so no masking/select ops are needed on device).
 - Words are sorted by length (desc) on host; step t processes exactly
   the alive prefix, rounded up to 64 (shared across cores via max).
 - State lives as [H=128 partitions, word], bf16: matmuls are
   weights-stationary with zero transposes; DVE elementwise ops run in
   2x mode; activations are one wide sigmoid over the [i|f|o] PSUM
   banks + tanh(g) + tanh(c), with tanh(c)/h clipped to the next step's
   alive width.
 - Tail: transpose c to [word, H] (PE), bf16 round-trip through DRAM
   with an indirect-DMA gather to undo the sort, raw pair dots + self
   dots (fused square+accumulate on ACT), cosine normalization after
   the dots via ln/exp, then the 2x2 convs + scorer lowered to tiny
   host-built matmuls.
"""

import os
import sys

for _p in ("/opt/trn_rl_repo",):
    if _p not in sys.path and os.path.isdir(_p):
        sys.path.insert(0, _p)

import ml_dtypes
import numpy as np

import concourse.bass as bass
import concourse.mybir as mybir
import concourse.tile as tile
from concourse.bass_utils import run_bass_kernel_spmd
from concourse.masks import make_identity

# This container's walrus build rejects CTRL instructions (Drain) carrying
# more than 2 sync waits ("Too many sync wait commands" in setupSyncWait).
# Tile's kernel-tail drain accumulates one wait per engine/DMA-queue sem, so
# redistribute: keep one wait on the drain, move the rest onto nofuse NOPs
# that execute before the all-engine barrier. Semantics are unchanged (all
# waits still complete before the barrier / semaphore teardown).
def _patched_drain_and_barrier(self, tick_clock, wait_clock):
    nc = self.nc
    drain_inst = nc.sync.drain()
    wait_clock.add_sem_waits(
        drain_inst.ins, tile.ScopedClock({None: tick_clock.global_clock})
    )
    waits = list(drain_inst.ins.sync_info.on_wait)
    if len(waits) > 1:
        drain_inst.ins.sync_info.on_wait = waits[:1]
        for k in range(1, len(waits)):
            nop = nc.sync.nop(nofuse=True, hint="drain_wait_spill")
            if nop.ins.sync_info is None:
                nop.ins.sync_info = mybir.SyncInfo(on_wait=[], on_update=[])
            nop.ins.sync_info.on_wait = [waits[k]]
    nc.all_engine_barrier()
    assert self.sems is not None
    popped = nc._tile_sem_poison_stack.pop()
    assert popped is self._sem_poison
    nc.clear_and_free_semaphores(list(self.sems.allocated().values()))
    nc.all_engine_barrier()


tile.TileContext._drain_and_barrier = _patched_drain_and_barrier

def _spill_excess_waits(nc):
    """Walrus here rejects instructions with more than ~2 sync waits. Spill
    excess waits onto same-engine NoOps inserted just before the instruction
    (engines dispatch in program order, so waiting earlier on the same engine
    is equivalent)."""
    cnt = [0]
    for fn in nc.m.functions:
        for bb in fn.blocks:
            insts = list(bb.instructions)
            out = []
            changed = False
            for inst in insts:
                si = inst.sync_info
                waits = list(si.on_wait) if si is not None and si.on_wait else []
                max_waits = 1
                if len(waits) > max_waits:
                    changed = True
                    keep = waits[-max_waits:]
                    extra = waits[:-max_waits]
                    for j in range(0, len(extra), max_waits):
                        cnt[0] += 1
                        nop = mybir.InstNoOp(name=f"I-spillw-{cnt[0]}", ins=[], outs=[])
                        nop.engine = inst.engine
                        nop.sync_info = mybir.SyncInfo(
                            on_wait=extra[j:j + max_waits], on_update=[])
                        nop.bass_nofuse = True
                        nop.bass_priority = 0
                        nop.text_hint = "spillw"
                        nop.debug = inst.debug
                        out.append(nop)
                    si.on_wait = keep
                out.append(inst)
            if changed:
                bb.instructions = out

B, NW, L, E, H, V = 8192, 4, 16, 128, 128, 64
NCORES = 8
PER = B // NCORES          # 1024 examples per core
NWORD = PER * NW           # 4096 words per core
NBLK = NWORD // 512        # 8 blocks of 512 words
NEC = PER // 128           # 8 example-chunks of 128
BLK = 512
FB = 30.0                  # freeze bias magnitude
F32 = mybir.dt.float32
BF16 = mybir.dt.bfloat16
AF = mybir.ActivationFunctionType
ALU = mybir.AluOpType

P6 = [(0, 1), (0, 2), (0, 3), (1, 2), (1, 3), (2, 3)]
# torch LSTMCell gate order is i,f,g,o; we pack PSUM banks as i,f,o,g so
# one wide sigmoid covers banks 0-2 and tanh covers bank 3.
GPERM = [0, 1, 3, 2]       # new gate m <- torch gate GPERM[m]


# ----------------------------------------------------------------- host prep

def _build_consts(inp):
    emb = np.asarray(inp["emb_i"], np.float32)
    W_ih = np.asarray(inp["W_ih"], np.float32)
    W_hh = np.asarray(inp["W_hh"], np.float32)
    b = np.asarray(inp["b_ih"], np.float32) + np.asarray(inp["b_hh"], np.float32)
    G = emb @ W_ih.T + b                       # [V, 4H] torch gate order
    G65 = np.zeros((V + 2, 4 * H), np.float32)
    WhhT = np.zeros((H, 4 * H), np.float32)
    for m, g in enumerate(GPERM):
        # gate bank m=3 is g: preacts doubled so tanh(z) = 2*sigmoid(2z)-1
        sc = 2.0 if m == 3 else 1.0
        G65[:V, m * H:(m + 1) * H] = sc * G[:, g * H:(g + 1) * H]
        WhhT[:, m * H:(m + 1) * H] = sc * W_hh[g * H:(g + 1) * H, :].T
    G65[V, 0:H] = -FB                          # i -> 0
    G65[V, H:2 * H] = +FB                      # f -> 1

    w1 = np.asarray(inp["conv1_w"], np.float32)
    b1 = np.asarray(inp["conv1_b"], np.float32)
    w2 = np.asarray(inp["conv2_w"], np.float32)
    b2 = np.asarray(inp["conv2_b"], np.float32)
    ws = np.asarray(inp["scorer_w"], np.float32)
    bs = float(np.asarray(inp["scorer_b"], np.float32)[0])

    p6idx = {p: i for i, p in enumerate(P6)}
    W1eff = np.zeros((6, 36), np.float32)
    b1eff = np.zeros((36, 1), np.float32)
    for c in range(4):
        for y in range(3):
            for x in range(3):
                m = c * 9 + y * 3 + x
                b1eff[m, 0] += b1[c]
                for dy in range(2):
                    for dx in range(2):
                        a, bb = y + dy, x + dx
                        w = w1[c, 0, dy, dx]
                        if a == bb:
                            b1eff[m, 0] += w
                        else:
                            W1eff[p6idx[(min(a, bb), max(a, bb))], m] += w
    W2eff = np.zeros((36, 32), np.float32)
    b2eff = np.zeros((32, 1), np.float32)
    for c2 in range(8):
        for y in range(2):
            for x in range(2):
                m = c2 * 4 + y * 2 + x
                b2eff[m, 0] = b2[c2]
                for c1 in range(4):
                    for dy in range(2):
                        for dx in range(2):
                            W2eff[c1 * 9 + (y + dy) * 3 + (x + dx), m] += w2[c2, c1, dy, dx]
    Wsc = ws[0].astype(np.float32).reshape(32, 1)
    return dict(G65=G65, WhhT=WhhT, W1eff=W1eff, b1eff=b1eff,
                W2eff=W2eff, b2eff=b2eff, Wsc=Wsc, bsc=bs)


def _core_prep(word_ids_c, lengths_c):
    wid = np.asarray(word_ids_c).reshape(NWORD, L)
    lens = np.asarray(lengths_c).reshape(NWORD)
    perm = np.argsort(-lens, kind="stable")
    inv = np.empty(NWORD, np.int32)
    inv[perm] = np.arange(NWORD, dtype=np.int32)
    wid_s = wid[perm]
    lens_s = lens[perm]
    Nt = (np.arange(L)[:, None] < lens_s[None, :]).sum(1)  # [L]
    return wid_s, lens_s, Nt, inv


def _build_onehot(wid_s, lens_s, widths):
    # 128 partition rows: 0..V-1 one-hot, V freeze flag, V+1..127 zero pad
    oh = np.zeros((L, 128, NWORD), np.float32)
    cols = np.arange(NWORD)
    for t in range(L):
        n = int(widths[t])
        if n == 0:
            continue
        alive = lens_s[:n] > t
        oh[t, wid_s[:n, t], cols[:n]] = alive.astype(np.float32)
        oh[t, V, cols[:n]] = (~alive).astype(np.float32)
    return oh


def _build_idx(inv):
    # idx[p, i*NEC + ec] = sorted-position of original word 4*(ec*128+p)+i
    idx = np.empty((128, NW * NEC), np.int32)
    p = np.arange(128)
    for i in range(NW):
        for ec in range(NEC):
            idx[:, i * NEC + ec] = inv[NW * (ec * 128 + p) + i]
    return idx


# -------------------------------------------------------------- bass program

def _bw(wt, k):
    """width of block k at a step of total width wt"""
    return max(0, min(BLK, wt - k * BLK))


# packed small-const blob layout (bf16 columns)
BLOB_C = 140
BLOB_IDX = 0      # [128, 64] bf16 = [128, 32] int32
BLOB_W1 = 64      # [6, 36] bf16
BLOB_W2 = 100     # [36, 32] bf16
BLOB_WSC = 132    # [32, 1] bf16
BLOB_B1 = 134     # [36, 2] bf16 = [36, 1] f32
BLOB_B2 = 136
BLOB_BSC = 138


def _retire_step(widths, k):
    """last step index that processes block k"""
    last = -1
    for t in range(L):
        if widths[t] > k * BLK:
            last = t
    return last


def _build_program(widths):
    """widths: tuple of per-step alive widths (len L, multiples of 64)."""
    nc = bass.Bass()

    FP8 = mybir.dt.float8e4
    oh_in = nc.dram_tensor("oh", [L, 128, NWORD], FP8, kind="ExternalInput")
    w8_in = nc.dram_tensor("w8", [128, 2, 4 * H], FP8, kind="ExternalInput")
    blob_in = nc.dram_tensor("blob", [128, BLOB_C], BF16, kind="ExternalInput")
    out_d = nc.dram_tensor("out", [1, PER], F32, kind="ExternalOutput")
    c_dram = nc.dram_tensor("cscratch", [NWORD, H], BF16)

    nsteps = [int(np.ceil(w / BLK)) for w in widths]
    # chunk-granular retirement: chunk c (words 128c..128c+127) is final
    # after the last step with width > 128c
    retire = {}
    for c in range(4 * NBLK):
        last = -1
        for t in range(L):
            if widths[t] > c * 128:
                last = t
        retire.setdefault(last, []).append(c)

    with tile.TileContext(nc) as tc:
        with (
            tc.tile_pool(name="const", bufs=1) as cpool,
            tc.tile_pool(name="state", bufs=1) as spool,
        ):
            # moving-operand ping-pong: [:, 0, :] one-hot (DMA'd per step),
            # [:, 1, :] recurrent h (written by DVE for the next step)
            ohh = [cpool.tile([128, 2, NWORD], FP8, tag=f"ohh{p}", name=f"ohh{p}")
                   for p in range(2)]
            # critical path to the first matmul: weights, then the first
            # 512 columns of the step-0 one-hot
            w8_sb = cpool.tile([128, 2, 4 * H], FP8, tag="w8", name="w8")
            nc.sync.dma_start(w8_sb[:], w8_in[:])
            nc.sync.dma_start(ohh[0][:, 0, :BLK], oh_in[0, :, :BLK])
            nc.sync.dma_start(ohh[0][:, 0, BLK:widths[0]], oh_in[0, :, BLK:widths[0]])
            blob_sb = cpool.tile([128, BLOB_C], BF16, tag="blob", name="blob")
            nc.sync.dma_start(blob_sb[:], blob_in[:])
            # h is zero at step 0
            nc.vector.memset(ohh[0][:, 1, :], 0.0)

            idx_sb = blob_sb[:, BLOB_IDX:BLOB_IDX + 64].bitcast(mybir.dt.int32)
            w1_sb = blob_sb[0:6, BLOB_W1:BLOB_W1 + 36]
            w2_sb = blob_sb[0:36, BLOB_W2:BLOB_W2 + 32]
            wsc_sb = blob_sb[0:32, BLOB_WSC:BLOB_WSC + 1]
            b1_sb = blob_sb[0:36, BLOB_B1:BLOB_B1 + 2].bitcast(F32)
            b2_sb = blob_sb[0:32, BLOB_B2:BLOB_B2 + 2].bitcast(F32)
            bsc_sb = blob_sb[0:1, BLOB_BSC:BLOB_BSC + 2].bitcast(F32)

            ident = cpool.tile([128, 128], F32, tag="ident", name="ident")

            # persistent cell state, one contiguous tile so multi-block
            # slices stay affine; t=0 writes every lane (no memsets)
            c_all = spool.tile([H, NBLK, BLK], BF16, tag="call", name="call")
            c_t = [c_all[:, k, :] for k in range(NBLK)]
            # [word, H] staging for the DRAM round trip, filled per retired block
            cT = spool.tile([128, NBLK * 4, H], BF16, tag="cT", name="cT")

            # ------------------------------------------------ LSTM main loop
            with (
                tc.tile_pool(name="gpsum", bufs=2, space="PSUM") as gpsum,
                tc.tile_pool(name="gsb", bufs=4) as gsb,
            ):
                for t in range(L):
                    wt = widths[t]
                    if wt == 0:
                        continue
                    wn = widths[t + 1] if t + 1 < L else 0
                    nb = nsteps[t]
                    cur, nxt = ohh[t % 2], ohh[(t + 1) % 2]
                    # prefetch the NEXT step's one-hot first so it is not
                    # queued behind this step's retirement transposes on Sync
                    if t + 1 < L and widths[t + 1] > 0:
                        nc.sync.dma_start(nxt[:, 0, :widths[t + 1]],
                                          oh_in[t + 1, :, :widths[t + 1]])
                    # split blocks into pieces at narrow steps so the serial
                    # mm->sigma->DVE->tanh->h chain still pipelines
                    pieces = []
                    split = BLK if nb >= 3 else (256 if nb == 2 else 128)
                    for k in range(nb):
                        w = _bw(wt, k)
                        wh = _bw(wn, k)
                        for off in range(0, w, split):
                            wp = min(split, w - off)
                            whp = min(max(wh - off, 0), wp)
                            pieces.append((k, off, wp, whp))
                    pend = None  # (block, o-gate AP) awaiting a paired tanh(c)
                    for pi, (k, off, w, wh) in enumerate(pieces):
                        ps = gpsum.tile([128, 4 * BLK], F32, tag="ps", name="ps")
                        rhs = cur[:, :, k * BLK + off:k * BLK + off + w]
                        for m in range(4):
                            sl = slice(m * BLK, m * BLK + w)
                            if w >= 128:
                                nc.tensor.matmul(
                                    ps[:, sl], lhsT=w8_sb[:, :, m * H:(m + 1) * H],
                                    rhs=rhs, start=True, stop=True,
                                    perf_mode=mybir.MatmulPerfMode.DoubleRow)
                            else:
                                nc.tensor.matmul(ps[:, sl], lhsT=w8_sb[:, 0, m * H:(m + 1) * H],
                                                 rhs=rhs[:, 0, :], start=True, stop=False)
                                nc.tensor.matmul(ps[:, sl], lhsT=w8_sb[:, 1, m * H:(m + 1) * H],
                                                 rhs=rhs[:, 1, :], start=False, stop=True)
                        # one sigmoid over all four gate banks (g preacts are
                        # doubled on host: tanh(z) = 2*sigmoid(2z) - 1)
                        sig = gsb.tile([128, 4 * BLK], BF16, tag="sig", name="sig")
                        sview = lambda ap, wdt: bass.AP(ap.tensor, ap.offset,
                                                        [ap.ap[0], [BLK, 4], [1, wdt]])
                        nc.scalar.activation(sview(sig, w), sview(ps, w), AF.Sigmoid)
                        gg = gsb.tile([128, BLK], BF16, tag="gg", name="gg")
                        nc.gpsimd.tensor_scalar(gg[:, :w], sig[:, 3 * BLK:3 * BLK + w],
                                                2.0, -1.0, ALU.mult, ALU.add)
                        ti = sig[:, 0:w]
                        tf = sig[:, BLK:BLK + w]
                        to = sig[:, 2 * BLK:2 * BLK + wh] if wh else None
                        cs = c_t[k][:, off:off + w]
                        ig = gsb.tile([128, BLK], BF16, tag="ig", name="ig")
                        if t == 0:
                            nc.vector.tensor_mul(cs, ti, gg[:, :w])
                        else:
                            nc.vector.tensor_mul(ig[:, :w], ti, gg[:, :w])
                            nc.vector.tensor_mul(cs, tf, cs)
                            nc.vector.tensor_add(cs, cs, ig[:, :w])
                        if wh:
                            if pend is not None:
                                # paired tanh over this block and the previous
                                # (full-width) one: contiguous in c_all
                                pk, pto = pend
                                pend = None
                                tt2 = gsb.tile([128, 2 * BLK], BF16, tag="tt2", name="tt2")
                                base = c_t[pk]
                                cpair = bass.AP(base.tensor, base.offset,
                                                [base.ap[0], [1, BLK + wh]])
                                nc.scalar.activation(tt2[:, :BLK + wh], cpair, AF.Tanh)
                                nc.vector.tensor_mul(
                                    nxt[:, 1, pk * BLK:pk * BLK + BLK],
                                    pto, tt2[:, :BLK])
                                nc.vector.tensor_mul(
                                    nxt[:, 1, k * BLK:k * BLK + wh],
                                    to, tt2[:, BLK:BLK + wh])
                            elif (off == 0 and wh == BLK and pi + 1 < len(pieces)
                                  and pieces[pi + 1][0] == k + 1
                                  and pieces[pi + 1][3] > 0):
                                pend = (k, to)
                            else:
                                tt = gsb.tile([128, BLK], BF16, tag="tt", name="tt")
                                nc.scalar.activation(tt[:, :wh], c_t[k][:, off:off + wh], AF.Tanh)
                                nc.vector.tensor_mul(
                                    nxt[:, 1, k * BLK + off:k * BLK + off + wh],
                                    to, tt[:, :wh])
                    if pend is not None:
                        pk, pto = pend
                        tt = gsb.tile([128, BLK], BF16, tag="tt", name="tt")
                        nc.scalar.activation(tt[:], c_t[pk][:], AF.Tanh)
                        nc.vector.tensor_mul(
                            nxt[:, 1, pk * BLK:(pk + 1) * BLK], pto, tt[:])
                    # retired chunks go to DRAM via xbar transpose + write,
                    # hidden under the remaining LSTM steps.  Deferred one
                    # (transpose) / two (write) steps so their semaphore
                    # waits are satisfied before they reach the in-order
                    # Sync engine and can't block the oh prefetch DMAs.
                    # Late retirees (t>=13) pile up at the end where short
                    # steps can't hide the Sync triggers; they go through PE
                    # transposes post-LSTM instead.
                    for ch in sorted(retire.get(t - 1, [])) if t - 1 < 13 else []:
                        k, j = ch // 4, ch % 4
                        nc.sync.dma_start_transpose(
                            cT[:, ch, :], c_t[k][:, j * 128:(j + 1) * 128])
                    for ch in sorted(retire.get(t - 2, [])) if t - 2 < 13 else []:
                        nc.sync.dma_start(
                            c_dram[128 * ch:128 * (ch + 1), :], cT[:, ch, :])

            # ------------------------------------------------------- tail
            with (
                tc.tile_pool(name="big", bufs=1) as big,
                tc.tile_pool(name="tpsum", bufs=2, space="PSUM") as tpsum,
                tc.tile_pool(name="cpsum", bufs=1, space="PSUM") as cpsum,
                tc.tile_pool(name="small", bufs=1) as small,
                tc.tile_pool(name="scr", bufs=2) as scrp,
            ):
                # undo the sort: per (ec) gather the example-chunk's 4 word
                # groups, then immediately normalize each word (fused
                # square+accum on ACT, ln/exp, scale) and compute cosine dots
                # (DVE) so compute hides under the gather stream.
                # late-retiring chunks: PE transpose -> ACT copy -> write
                late = sorted(ch for t, chs in retire.items() if t >= 13
                              for ch in chs)
                make_identity(nc, ident[:])
                identb = small.tile([128, 128], BF16, tag="identb", name="identb")
                nc.vector.tensor_copy(identb[:], ident[:])
                for ch in late:
                    k, j = ch // 4, ch % 4
                    ptc = tpsum.tile([128, 128], BF16, tag="ptc", name="ptc")
                    nc.tensor.transpose(ptc[:], c_t[k][:, j * 128:(j + 1) * 128],
                                        identb[:])
                    nc.scalar.copy(cT[:, ch, :], ptc[:])
                    nc.sync.dma_start(
                        c_dram[128 * ch:128 * (ch + 1), :], cT[:, ch, :])

                A = big.tile([128, NW * NEC, H], BF16, tag="A", name="A")
                d2 = small.tile([128, NW * NEC], F32, tag="d2", name="d2")
                s_all = small.tile([128, NW * NEC], F32, tag="s", name="s")
                Dp = small.tile([128, 6 * NEC], F32, tag="Dp", name="Dp")
                sqs = scrp.tile([128, H], F32, tag="sqs", name="sqs")
                cos6 = small.tile([6, PER], BF16, tag="cos6", name="cos6")
                ecv = lambda ap, ec: bass.AP(ap.tensor, ap.offset + ec,
                                             [ap.ap[0], [NEC, NW]])
                for ec in range(NEC):
                    for i in range(NW):
                        b = i * NEC + ec
                        nc.gpsimd.indirect_dma_start(
                            out=A[:, b, :],
                            out_offset=None,
                            in_=c_dram[:],
                            in_offset=bass.IndirectOffsetOnAxis(
                                ap=idx_sb[:, b:b + 1], axis=0),
                        )
                    for i in range(NW):
                        b = i * NEC + ec
                        nc.scalar.activation(sqs[:], A[:, b, :], AF.Square,
                                             accum_out=d2[:, b:b + 1])
                    # s = 1/sqrt(d2) via ln/exp on the ec-strided 4-col views
                    nc.vector.tensor_scalar_max(ecv(d2, ec), ecv(d2, ec), 1e-30)
                    nc.scalar.activation(ecv(s_all, ec), ecv(d2, ec), AF.Ln)
                    nc.scalar.activation(ecv(s_all, ec), ecv(s_all, ec), AF.Exp,
                                         scale=-0.5)
                    for i in range(NW):
                        b = i * NEC + ec
                        nc.vector.tensor_scalar_mul(A[:, b, :], A[:, b, :],
                                                    s_all[:, b:b + 1])
                    for k, (i, j) in enumerate(P6):
                        scr = scrp.tile([128, H], BF16, tag="scr", name="scr")
                        nc.vector.tensor_mul(scr[:], A[:, i * NEC + ec, :],
                                             A[:, j * NEC + ec, :])
                        nc.vector.tensor_reduce(
                            Dp[:, k * NEC + ec:k * NEC + ec + 1],
                            scr[:], axis=mybir.AxisListType.X, op=ALU.add)
                    pt6 = tpsum.tile([128, 128], F32, tag="tp6", name="tp6")
                    dview = bass.AP(Dp.tensor, Dp.offset + ec,
                                    [Dp.ap[0], [NEC, 6]])
                    nc.tensor.transpose(pt6[:6, :], dview, ident[:])
                    nc.scalar.copy(cos6[:, ec * 128:(ec + 1) * 128], pt6[:6, :])

                    # run the scorer per 256-example quarter so earlier
                    # quarters overlap later gathers and the final chain
                    # after the last gather is short
                    if ec % 2 == 0:
                        continue
                    sl = slice((ec // 2) * 256, (ec // 2 + 1) * 256)
                    r1 = small.tile([36, 256], BF16, tag="r1", name="r1")
                    r2 = small.tile([32, 256], BF16, tag="r2", name="r2")
                    o_sb = small.tile([1, 256], F32, tag="o", name="o")
                    p1 = cpsum.tile([36, 256], F32, tag="cp1", name="cp1")
                    nc.tensor.matmul(p1[:], lhsT=w1_sb[:], rhs=cos6[:, sl],
                                     start=True, stop=True)
                    nc.scalar.activation(r1[:], p1[:], AF.Relu, bias=b1_sb[:, 0:1])
                    p2 = cpsum.tile([32, 256], F32, tag="cp2", name="cp2")
                    nc.tensor.matmul(p2[:], lhsT=w2_sb[:], rhs=r1[:],
                                     start=True, stop=True)
                    nc.scalar.activation(r2[:], p2[:], AF.Relu, bias=b2_sb[:, 0:1])
                    p3 = cpsum.tile([1, 256], F32, tag="cp3", name="cp3")
                    nc.tensor.matmul(p3[:], lhsT=wsc_sb[:], rhs=r2[:],
                                     start=True, stop=True)
                    nc.scalar.activation(o_sb[:], p3[:], AF.Sigmoid,
                                         bias=bsc_sb[0:1, 0:1])
                    nc.sync.dma_start(out_d[:, sl], o_sb[:])

    return nc


_prog_cache = {}


def _get_program(widths):
    key = tuple(int(x) for x in widths)
    if key not in _prog_cache:
        _prog_cache[key] = _build_program(key)
    return _prog_cache[key]


def _run(inputs, trace=False):
    consts = _build_consts(inputs)
    word_ids = np.asarray(inputs["word_ids"])
    lengths = np.asarray(inputs["lengths"])

    preps = []
    for c in range(NCORES):
        sl = slice(c * PER, (c + 1) * PER)
        preps.append(_core_prep(word_ids[sl], lengths[sl]))
    Nt_max = np.stack([p[2] for p in preps]).max(0)
    widths = tuple(int(x) for x in (np.ceil(Nt_max / 64) * 64).astype(np.int64))

    fp8 = ml_dtypes.float8_e4m3
    w8 = np.zeros((128, 2, 4 * H), np.float32)
    w8[:V + 2, 0, :] = consts["G65"]
    w8[:, 1, :] = consts["WhhT"]
    w8 = np.clip(w8, -240, 240).astype(fp8)

    def _pack_blob(idx):
        blob = np.zeros((128, BLOB_C), ml_dtypes.bfloat16)
        blob[:, BLOB_IDX:BLOB_IDX + 64] = idx.view(np.uint16).view(ml_dtypes.bfloat16)
        blob[0:6, BLOB_W1:BLOB_W1 + 36] = consts["W1eff"].astype(ml_dtypes.bfloat16)
        blob[0:36, BLOB_W2:BLOB_W2 + 32] = consts["W2eff"].astype(ml_dtypes.bfloat16)
        blob[0:32, BLOB_WSC:BLOB_WSC + 1] = consts["Wsc"].astype(ml_dtypes.bfloat16)
        blob[0:36, BLOB_B1:BLOB_B1 + 2] = consts["b1eff"].astype(np.float32).view(np.uint16).view(ml_dtypes.bfloat16)
        blob[0:32, BLOB_B2:BLOB_B2 + 2] = consts["b2eff"].astype(np.float32).view(np.uint16).view(ml_dtypes.bfloat16)
        blob[0:1, BLOB_BSC:BLOB_BSC + 2] = np.full((1, 1), consts["bsc"], np.float32).view(np.uint16).view(ml_dtypes.bfloat16)
        return blob

    in_maps = []
    for c in range(NCORES):
        wid_s, lens_s, _, inv = preps[c]
        in_maps.append({
            "oh": _build_onehot(wid_s, lens_s, widths).astype(fp8),
            "w8": w8,
            "blob": _pack_blob(_build_idx(inv)),
        })

    nc = _get_program(widths)
    _spill_excess_waits(nc)  # idempotent; HW-compile only (CoreSim dislikes raw NoOps)
    res = run_bass_kernel_spmd(nc, in_maps, list(range(NCORES)), trace=trace)
    out = np.concatenate([np.asarray(r["out"]).reshape(PER) for r in res.results])
    return out.reshape(B, 1).astype(np.float32), res.exec_time_ns


def kernel(**inputs):
    return _run(inputs)[0]
